# revision 1
# baseline (speedup 1.0000x reference)
"""Trainium2 Bass kernel for nn_CriticReadoutModule (twin-critic GNN readout).

Computes, for each sample b:
  x_o   = concat(obs[b,:64], act[b], ctx[b,o])            # [264] per object
  h_o   = relu(x_o @ W1c + b1c)                           # [256] per critic c
  p     = sum_o relu(h_o @ W2c + b2c)                     # [192]
  q_c   = relu(p @ R1c + rb1c) @ R2c + rb2c               # scalar
Returns (q1, q2), each [B, 1] float32.

Strategy: pure data parallelism over batch across 8 cores. On-chip layout is
feature-major (features on SBUF partitions, batch on the free dim), which the
host arranges by pre-transposing ctx -> [O, 184, B] and body -> [80, B].
Matmuls run as float32r (full PE rate at N=512). The object-sum uses the
identity sum_o relu(y+b) = sum_o max(y,-b) + 10*b, with the 10*b correction
folded into rho's first-layer bias on the host, so the whole L2 epilogue is a
single fused DVE op per tile: acc = max(psum, -b2) + acc.
"""

import sys
import numpy as np

for _p in ("/opt/trn_rl_repo",):
    if _p not in sys.path:
        sys.path.insert(0, _p)

N_CORES = 8
DIM_BODY = 64
DIM_ACT = 16
DBODY = DIM_BODY + DIM_ACT  # 80
O = 10
DCTX = 184
HID = 256
DPHI = 192
NT = 512  # moving-dim tile (one fp32 PSUM bank)


def build_nc(S, repeat=1):
    """Build + compile the per-core Bass module for a shard of S samples.

    repeat>1 wraps the whole computation in a hardware loop that redoes it
    (on the same data) repeat times — used only for timing measurements.
    """
    import concourse.bacc as bacc
    import concourse.bass as bass
    import concourse.mybir as mybir
    import concourse.tile as tile

    f32 = mybir.dt.float32
    f32r = mybir.dt.float32r
    AF = mybir.ActivationFunctionType
    ALU = mybir.AluOpType

    assert S % NT == 0
    BT = S // NT

    nc = bacc.Bacc("TRN2", target_bir_lowering=False, debug=False)

    # ---- DRAM I/O ----
    body_d = nc.dram_tensor("body_t", [DBODY, S], f32r, kind="ExternalInput")
    ctx_d = nc.dram_tensor("ctx_t", [O, DCTX, S], f32r, kind="ExternalInput")
    w1_d = nc.dram_tensor("w1s", [DBODY + DCTX, 2 * HID], f32r, kind="ExternalInput")
    w2a_d = nc.dram_tensor("w2a", [HID, DPHI], f32r, kind="ExternalInput")
    w2b_d = nc.dram_tensor("w2b", [HID, DPHI], f32r, kind="ExternalInput")
    rw1a_d = nc.dram_tensor("rw1a", [DPHI, HID], f32r, kind="ExternalInput")
    rw1b_d = nc.dram_tensor("rw1b", [DPHI, HID], f32r, kind="ExternalInput")
    rw2a_d = nc.dram_tensor("rw2a", [HID, 1], f32r, kind="ExternalInput")
    rw2b_d = nc.dram_tensor("rw2b", [HID, 1], f32r, kind="ExternalInput")
    b1_d = nc.dram_tensor("b1s", [4, 128], f32, kind="ExternalInput")
    nb2_128_d = nc.dram_tensor("nb2_128", [2, 128], f32, kind="ExternalInput")
    nb2_64_d = nc.dram_tensor("nb2_64", [2, 64], f32, kind="ExternalInput")
    rb1_d = nc.dram_tensor("rb1adj", [4, 128], f32, kind="ExternalInput")
    rb2_d = nc.dram_tensor("rb2", [2, 1], f32, kind="ExternalInput")
    q1_d = nc.dram_tensor("q1", [S, 1], f32, kind="ExternalOutput")
    q2_d = nc.dram_tensor("q2", [S, 1], f32, kind="ExternalOutput")

    with tile.TileContext(nc) as tc:
        with (
            tc.tile_pool(name="wp", bufs=1) as wp,
            tc.tile_pool(name="dp", bufs=1) as dp,
            tc.tile_pool(name="pp", bufs=1, space="PSUM") as pp,
        ):
            # ---- static weights / biases (loaded once) ----
            w_body = wp.tile([DBODY, 2 * HID], f32r, name="w_body")
            w_chi = wp.tile([128, 2 * HID], f32r, name="w_chi")
            w_clo = wp.tile([DCTX - 128, 2 * HID], f32r, name="w_clo")
            nc.sync.dma_start(w_body[:], w1_d[0:DBODY, :])
            nc.sync.dma_start(w_chi[:], w1_d[DBODY : DBODY + 128, :])
            nc.sync.dma_start(w_clo[:], w1_d[DBODY + 128 :, :])

            w2 = {}
            for cn, wd in (("a", w2a_d), ("b", w2b_d)):
                for k in range(2):
                    t = wp.tile([128, DPHI], f32r, name=f"w2{cn}k{k}")
                    nc.sync.dma_start(t[:], wd[k * 128 : (k + 1) * 128, :])
                    w2[cn, k] = t
            rw1 = {}
            for cn, wd in (("a", rw1a_d), ("b", rw1b_d)):
                t0 = wp.tile([128, HID], f32r, name=f"rw1{cn}0")
                t1 = wp.tile([DPHI - 128, HID], f32r, name=f"rw1{cn}1")
                nc.sync.dma_start(t0[:], wd[0:128, :])
                nc.sync.dma_start(t1[:], wd[128:DPHI, :])
                rw1[cn] = (t0, t1)
            rw2 = {}
            for cn, wd in (("a", rw2a_d), ("b", rw2b_d)):
                t0 = wp.tile([128, 1], f32r, name=f"rw2{cn}0")
                t1 = wp.tile([128, 1], f32r, name=f"rw2{cn}1")
                nc.sync.dma_start(t0[:], wd[0:128, :])
                nc.sync.dma_start(t1[:], wd[128:HID, :])
                rw2[cn] = (t0, t1)

            b1c = []
            for m in range(4):
                t = wp.tile([128, 1], f32, name=f"b1c{m}")
                nc.sync.dma_start(t[:], b1_d[m, :])
                b1c.append(t)
            nb2 = {}
            for i, cn in enumerate(("a", "b")):
                t0 = wp.tile([128, 1], f32, name=f"nb2{cn}0")
                nc.sync.dma_start(t0[:], nb2_128_d[i, :])
                t1 = wp.tile([64, 1], f32, name=f"nb2{cn}1")
                nc.sync.dma_start(t1[:], nb2_64_d[i, :])
                nb2[cn] = (t0, t1)
            rb1c = []
            for m in range(4):
                t = wp.tile([128, 1], f32, name=f"rb1c{m}")
                nc.sync.dma_start(t[:], rb1_d[m, :])
                rb1c.append(t)
            rb2c = []
            for i in range(2):
                t = wp.tile([1, 1], f32, name=f"rb2c{i}")
                nc.sync.dma_start(t[:], rb2_d[i, :])
                rb2c.append(t)

            # body activations for the whole shard (tiny: 80 x S)
            body_sb = wp.tile([DBODY, S], f32r, name="body_sb")
            nc.sync.dma_start(body_sb[:], body_d[:])

            # ---- main loop ----
            import contextlib

            rep_ctx = tc.For_i(0, repeat, 1) if repeat > 1 else contextlib.nullcontext()
            with rep_ctx:
                _main_body(
                    nc, tc, dp, pp, BT, f32, f32r, AF, ALU,
                    ctx_d, q1_d, q2_d, body_sb,
                    w_body, w_chi, w_clo, w2, rw1, rw2,
                    b1c, nb2, rb1c, rb2c,
                )

    nc.compile()
    return nc


def _main_body(nc, tc, dp, pp, BT, f32, f32r, AF, ALU, ctx_d, q1_d, q2_d, body_sb,
               w_body, w_chi, w_clo, w2, rw1, rw2, b1c, nb2, rb1c, rb2c):
    S = BT * NT
    DCTX_ = DCTX
    if True:
            for bt in range(BT):
                bs = slice(bt * NT, (bt + 1) * NT)
                # object-sum accumulators (fp32, feature-major)
                acc = {
                    "a": (
                        dp.tile([128, NT], f32r, name="acA0", tag="acc128", bufs=4),
                        dp.tile([64, NT], f32r, name="acA1", tag="acc64", bufs=4),
                    ),
                    "b": (
                        dp.tile([128, NT], f32r, name="acB0", tag="acc128", bufs=4),
                        dp.tile([64, NT], f32r, name="acB1", tag="acc64", bufs=4),
                    ),
                }
                for o in range(O):
                    c_hi = dp.tile([128, NT], f32r, name="c_hi", tag="c_hi", bufs=3)
                    c_lo = dp.tile([DCTX - 128, NT], f32r, name="c_lo", tag="c_lo", bufs=3)
                    nc.sync.dma_start(c_hi[:], ctx_d[o, 0:128, bs])
                    nc.sync.dma_start(c_lo[:], ctx_d[o, 128:DCTX, bs])

                    # L1: h = relu(x @ W1 + b1), stacked critics -> 4 M-chunks
                    h = []
                    for m in range(4):
                        ms = slice(m * 128, (m + 1) * 128)
                        ph = pp.tile([128, NT], f32, name="ph", tag="ph", bufs=3)
                        nc.tensor.matmul(ph[:], w_body[:, ms], body_sb[:, bs], start=True, stop=False)
                        nc.tensor.matmul(ph[:], w_chi[:, ms], c_hi[:], start=False, stop=False)
                        nc.tensor.matmul(ph[:], w_clo[:, ms], c_lo[:], start=False, stop=True)
                        ht = dp.tile([128, NT], f32r, name="h", tag="h", bufs=8)
                        nc.scalar.activation(ht[:], ph[:], AF.Relu, bias=b1c[m][:])
                        h.append(ht)

                    # L2 + fused object-sum: acc = max(psum, -b2) + acc
                    for cn, h0, h1 in (("a", h[0], h[1]), ("b", h[2], h[3])):
                        for part in range(2):
                            mslice = slice(0, 128) if part == 0 else slice(128, DPHI)
                            mp = 128 if part == 0 else DPHI - 128
                            ppt = pp.tile([mp, NT], f32, name="pp2", tag=f"pp2_{part}", bufs=2)
                            nc.tensor.matmul(ppt[:], w2[cn, 0][:, mslice], h0[:], start=True, stop=False)
                            nc.tensor.matmul(ppt[:], w2[cn, 1][:, mslice], h1[:], start=False, stop=True)
                            a = acc[cn][part]
                            nb = nb2[cn][part]
                            if o == 0:
                                nc.vector.tensor_scalar(a[:], ppt[:], nb[:], None, ALU.max)
                            else:
                                nc.vector.scalar_tensor_tensor(
                                    a[:], ppt[:], nb[:], a[:], op0=ALU.max, op1=ALU.add
                                )

                # rho: q_c = relu(p @ R1 + rb1') @ R2 + rb2
                for ci, cn in enumerate(("a", "b")):
                    a0, a1 = acc[cn]
                    t0, t1 = rw1[cn]
                    zr = []
                    for m in range(2):
                        ms = slice(m * 128, (m + 1) * 128)
                        psz = pp.tile([128, NT], f32, name="psz", tag="ph", bufs=3)
                        nc.tensor.matmul(psz[:], t0[:, ms], a0[:], start=True, stop=False)
                        nc.tensor.matmul(psz[:], t1[:, ms], a1[:], start=False, stop=True)
                        zt = dp.tile([128, NT], f32r, name="zr", tag="zr", bufs=4)
                        nc.scalar.activation(zt[:], psz[:], AF.Relu, bias=rb1c[2 * ci + m][:])
                        zr.append(zt)
                    psq = pp.tile([1, NT], f32, name="psq", tag="pp2_1", bufs=2)
                    nc.tensor.matmul(psq[:], rw2[cn][0][:], zr[0][:], start=True, stop=False)
                    nc.tensor.matmul(psq[:], rw2[cn][1][:], zr[1][:], start=False, stop=True)
                    qt = dp.tile([1, NT], f32, name="qt", tag="qt", bufs=4)
                    nc.scalar.activation(qt[:], psq[:], AF.Identity, bias=rb2c[ci][:])
                    qd = q1_d if ci == 0 else q2_d
                    nc.sync.dma_start(qd[bs, :], qt[:])


def prep_inputs(inputs, S, core):
    """Host-side shard + layout prep for one core. Returns the in_map."""
    lo, hi = core * S, (core + 1) * S
    obs = inputs["obs"][lo:hi]
    act = inputs["act"][lo:hi]
    ctx = inputs["context_layer"][lo:hi]
    body_t = np.ascontiguousarray(
        np.concatenate([obs[:, :DIM_BODY], act], axis=1).T
    ).astype(np.float32)
    ctx_t = np.ascontiguousarray(np.transpose(ctx, (1, 2, 0))).astype(np.float32)

    w1s = np.concatenate([inputs["phi_w1a"], inputs["phi_w1b"]], axis=1)
    b1s = np.concatenate([inputs["phi_b1a"], inputs["phi_b1b"]]).reshape(4, 128)
    b2a, b2b = inputs["phi_b2a"], inputs["phi_b2b"]
    nb2_128 = np.stack([-b2a[0:128], -b2b[0:128]])
    nb2_64 = np.stack([-b2a[128:DPHI], -b2b[128:DPHI]])
    # rho1 bias adjusted for the +O*b2 correction of the max-based object sum
    rb1a = inputs["rho_b1a"] + O * (b2a @ inputs["rho_w1a"])
    rb1b = inputs["rho_b1b"] + O * (b2b @ inputs["rho_w1b"])
    rb1adj = np.concatenate([rb1a, rb1b]).reshape(4, 128)
    rb2 = np.stack([inputs["rho_b2a"], inputs["rho_b2b"]]).reshape(2, 1)

    f = np.float32
    return {
        "body_t": body_t,
        "ctx_t": ctx_t,
        "w1s": np.ascontiguousarray(w1s, f),
        "w2a": np.ascontiguousarray(inputs["phi_w2a"], f),
        "w2b": np.ascontiguousarray(inputs["phi_w2b"], f),
        "rw1a": np.ascontiguousarray(inputs["rho_w1a"], f),
        "rw1b": np.ascontiguousarray(inputs["rho_w1b"], f),
        "rw2a": np.ascontiguousarray(inputs["rho_w2a"], f),
        "rw2b": np.ascontiguousarray(inputs["rho_w2b"], f),
        "b1s": np.ascontiguousarray(b1s, f),
        "nb2_128": np.ascontiguousarray(nb2_128, f),
        "nb2_64": np.ascontiguousarray(nb2_64, f),
        "rb1adj": np.ascontiguousarray(rb1adj, f),
        "rb2": np.ascontiguousarray(rb2, f),
    }


_CACHE = {}


def kernel(**inputs):
    from concourse.bass_utils import run_bass_kernel_spmd

    B = inputs["obs"].shape[0]
    assert B % N_CORES == 0
    S = B // N_CORES

    if S not in _CACHE:
        _CACHE[S] = build_nc(S)
    nc = _CACHE[S]

    in_maps = [prep_inputs(inputs, S, c) for c in range(N_CORES)]
    res = run_bass_kernel_spmd(nc, in_maps, list(range(N_CORES)))
    q1 = np.concatenate([res.results[c]["q1"] for c in range(N_CORES)], axis=0)
    q2 = np.concatenate([res.results[c]["q2"] for c in range(N_CORES)], axis=0)
    return (q1.astype(np.float32), q2.astype(np.float32))


if __name__ == "__main__":
    # smoke test with random data
    rng = np.random.default_rng(0)
    B = 32768
    ins = {
        "obs": rng.standard_normal((B, 100), dtype=np.float32),
        "act": rng.standard_normal((B, DIM_ACT), dtype=np.float32),
        "context_layer": rng.standard_normal((B, O, DCTX), dtype=np.float32),
    }
    for n, shp in (
        ("phi_w1a", (264, 256)), ("phi_b1a", (256,)),
        ("phi_w2a", (256, 192)), ("phi_b2a", (192,)),
        ("phi_w1b", (264, 256)), ("phi_b1b", (256,)),
        ("phi_w2b", (256, 192)), ("phi_b2b", (192,)),
        ("rho_w1a", (192, 256)), ("rho_b1a", (256,)),
        ("rho_w2a", (256, 1)), ("rho_b2a", (1,)),
        ("rho_w1b", (192, 256)), ("rho_b1b", (256,)),
        ("rho_w2b", (256, 1)), ("rho_b2b", (1,)),
    ):
        ins[n] = (rng.standard_normal(shp) * 0.05).astype(np.float32)
    q1, q2 = kernel(**ins)
    print(q1.shape, q2.shape, q1[:4, 0], q2[:4, 0])



# revision 2
# speedup vs baseline: 455.0417x; 455.0417x over previous
"""Trainium2 Bass kernel for nn_CriticReadoutModule (twin-critic GNN readout).

Computes, for each sample b:
  x_o   = concat(obs[b,:64], act[b], ctx[b,o])            # [264] per object
  h_o   = relu(x_o @ W1c + b1c)                           # [256] per critic c
  p     = sum_o relu(h_o @ W2c + b2c)                     # [192]
  q_c   = relu(p @ R1c + rb1c) @ R2c + rb2c               # scalar
Returns (q1, q2), each [B, 1] float32.

Strategy: pure data parallelism over batch across 8 cores. On-chip layout is
feature-major (features on SBUF partitions, batch on the free dim), which the
host arranges by pre-transposing ctx -> [O, 184, B] and body -> [80, B].
All matmul operands are bf16 (PSUM accumulation stays fp32): bf16 stationary
weights get the fast weight-load path on real HW, and bf16 halves HBM traffic.
The object-sum uses the identity sum_o relu(y+b) = sum_o max(y,-b) + 10*b,
with the 10*b correction folded into rho's first-layer bias on the host, so
the whole L2 epilogue is a single fused DVE op per tile:
acc = max(psum, -b2) + acc.
"""

import sys
import numpy as np

for _p in ("/opt/trn_rl_repo",):
    if _p not in sys.path:
        sys.path.insert(0, _p)

N_CORES = 8
DIM_BODY = 64
DIM_ACT = 16
DBODY = DIM_BODY + DIM_ACT  # 80
O = 10
DCTX = 184
HID = 256
DPHI = 192
NT = 512  # moving-dim tile (one fp32 PSUM bank)


def build_nc(S, repeat=1):
    """Build + compile the per-core Bass module for a shard of S samples.

    repeat>1 wraps the whole computation in a hardware loop that redoes it
    (on the same data) repeat times — used only for timing measurements.
    """
    import concourse.bacc as bacc
    import concourse.bass as bass
    import concourse.mybir as mybir
    import concourse.tile as tile

    f32 = mybir.dt.float32
    bf16 = mybir.dt.bfloat16
    AF = mybir.ActivationFunctionType
    ALU = mybir.AluOpType

    assert S % NT == 0
    BT = S // NT

    nc = bacc.Bacc("TRN2", target_bir_lowering=False, debug=False)

    # ---- DRAM I/O ----
    body_d = nc.dram_tensor("body_t", [DBODY, S], bf16, kind="ExternalInput")
    ctx_d = nc.dram_tensor("ctx_t", [O, DCTX, S], bf16, kind="ExternalInput")
    w1_d = nc.dram_tensor("w1s", [DBODY + DCTX, 2 * HID], bf16, kind="ExternalInput")
    w2a_d = nc.dram_tensor("w2a", [HID, DPHI], bf16, kind="ExternalInput")
    w2b_d = nc.dram_tensor("w2b", [HID, DPHI], bf16, kind="ExternalInput")
    rw1a_d = nc.dram_tensor("rw1a", [DPHI, HID], bf16, kind="ExternalInput")
    rw1b_d = nc.dram_tensor("rw1b", [DPHI, HID], bf16, kind="ExternalInput")
    rw2a_d = nc.dram_tensor("rw2a", [HID, 1], bf16, kind="ExternalInput")
    rw2b_d = nc.dram_tensor("rw2b", [HID, 1], bf16, kind="ExternalInput")
    b1_d = nc.dram_tensor("b1s", [4, 128], f32, kind="ExternalInput")
    nb2_128_d = nc.dram_tensor("nb2_128", [2, 128], f32, kind="ExternalInput")
    nb2_64_d = nc.dram_tensor("nb2_64", [2, 64], f32, kind="ExternalInput")
    rb1_d = nc.dram_tensor("rb1adj", [4, 128], f32, kind="ExternalInput")
    rb2_d = nc.dram_tensor("rb2", [2, 1], f32, kind="ExternalInput")
    q1_d = nc.dram_tensor("q1", [S, 1], f32, kind="ExternalOutput")
    q2_d = nc.dram_tensor("q2", [S, 1], f32, kind="ExternalOutput")

    with tile.TileContext(nc) as tc:
        with (
            tc.tile_pool(name="wp", bufs=1) as wp,
            tc.tile_pool(name="dp", bufs=1) as dp,
            tc.tile_pool(name="pp", bufs=1, space="PSUM") as pp,
        ):
            # ---- static weights / biases (loaded once) ----
            w_body = wp.tile([DBODY, 2 * HID], bf16, name="w_body")
            w_chi = wp.tile([128, 2 * HID], bf16, name="w_chi")
            w_clo = wp.tile([DCTX - 128, 2 * HID], bf16, name="w_clo")
            nc.sync.dma_start(w_body[:], w1_d[0:DBODY, :])
            nc.sync.dma_start(w_chi[:], w1_d[DBODY : DBODY + 128, :])
            nc.sync.dma_start(w_clo[:], w1_d[DBODY + 128 :, :])

            w2 = {}
            for cn, wd in (("a", w2a_d), ("b", w2b_d)):
                for k in range(2):
                    t = wp.tile([128, DPHI], bf16, name=f"w2{cn}k{k}")
                    nc.sync.dma_start(t[:], wd[k * 128 : (k + 1) * 128, :])
                    w2[cn, k] = t
            rw1 = {}
            for cn, wd in (("a", rw1a_d), ("b", rw1b_d)):
                t0 = wp.tile([128, HID], bf16, name=f"rw1{cn}0")
                t1 = wp.tile([DPHI - 128, HID], bf16, name=f"rw1{cn}1")
                nc.sync.dma_start(t0[:], wd[0:128, :])
                nc.sync.dma_start(t1[:], wd[128:DPHI, :])
                rw1[cn] = (t0, t1)
            rw2 = {}
            for cn, wd in (("a", rw2a_d), ("b", rw2b_d)):
                t0 = wp.tile([128, 1], bf16, name=f"rw2{cn}0")
                t1 = wp.tile([128, 1], bf16, name=f"rw2{cn}1")
                nc.sync.dma_start(t0[:], wd[0:128, :])
                nc.sync.dma_start(t1[:], wd[128:HID, :])
                rw2[cn] = (t0, t1)

            b1c = []
            for m in range(4):
                t = wp.tile([128, 1], f32, name=f"b1c{m}")
                nc.sync.dma_start(t[:], b1_d[m, :])
                b1c.append(t)
            nb2 = {}
            for i, cn in enumerate(("a", "b")):
                t0 = wp.tile([128, 1], f32, name=f"nb2{cn}0")
                nc.sync.dma_start(t0[:], nb2_128_d[i, :])
                t1 = wp.tile([64, 1], f32, name=f"nb2{cn}1")
                nc.sync.dma_start(t1[:], nb2_64_d[i, :])
                nb2[cn] = (t0, t1)
            rb1c = []
            for m in range(4):
                t = wp.tile([128, 1], f32, name=f"rb1c{m}")
                nc.sync.dma_start(t[:], rb1_d[m, :])
                rb1c.append(t)
            rb2c = []
            for i in range(2):
                t = wp.tile([1, 1], f32, name=f"rb2c{i}")
                nc.sync.dma_start(t[:], rb2_d[i, :])
                rb2c.append(t)

            # body activations for the whole shard (tiny: 80 x S)
            body_sb = wp.tile([DBODY, S], bf16, name="body_sb")
            nc.sync.dma_start(body_sb[:], body_d[:])

            # ---- main loop ----
            import contextlib

            rep_ctx = tc.For_i(0, repeat, 1) if repeat > 1 else contextlib.nullcontext()
            with rep_ctx:
                _main_body(
                    nc, tc, dp, pp, BT, f32, bf16, AF, ALU,
                    ctx_d, q1_d, q2_d, body_sb,
                    w_body, w_chi, w_clo, w2, rw1, rw2,
                    b1c, nb2, rb1c, rb2c,
                )

    nc.compile()
    return nc


def _main_body(nc, tc, dp, pp, BT, f32, bf16, AF, ALU, ctx_d, q1_d, q2_d, body_sb,
               w_body, w_chi, w_clo, w2, rw1, rw2, b1c, nb2, rb1c, rb2c):
    for bt in range(BT):
        bs = slice(bt * NT, (bt + 1) * NT)
        # object-sum accumulators (feature-major)
        acc = {
            "a": (
                dp.tile([128, NT], bf16, name="acA0", tag="acc128", bufs=4),
                dp.tile([64, NT], bf16, name="acA1", tag="acc64", bufs=4),
            ),
            "b": (
                dp.tile([128, NT], bf16, name="acB0", tag="acc128", bufs=4),
                dp.tile([64, NT], bf16, name="acB1", tag="acc64", bufs=4),
            ),
        }
        for o in range(O):
            c_hi = dp.tile([128, NT], bf16, name="c_hi", tag="c_hi", bufs=3)
            c_lo = dp.tile([DCTX - 128, NT], bf16, name="c_lo", tag="c_lo", bufs=3)
            nc.sync.dma_start(c_hi[:], ctx_d[o, 0:128, bs])
            nc.sync.dma_start(c_lo[:], ctx_d[o, 128:DCTX, bs])

            # L1: h = relu(x @ W1 + b1), stacked critics -> 4 M-chunks
            h = []
            for m in range(4):
                ms = slice(m * 128, (m + 1) * 128)
                ph = pp.tile([128, NT], f32, name="ph", tag="ph", bufs=4)
                nc.tensor.matmul(ph[:], w_body[:, ms], body_sb[:, bs], start=True, stop=False)
                nc.tensor.matmul(ph[:], w_chi[:, ms], c_hi[:], start=False, stop=False)
                nc.tensor.matmul(ph[:], w_clo[:, ms], c_lo[:], start=False, stop=True)
                ht = dp.tile([128, NT], bf16, name="h", tag="h", bufs=8)
                nc.scalar.activation(ht[:], ph[:], AF.Relu, bias=b1c[m][:])
                h.append(ht)

            # L2 + fused object-sum: acc = max(psum, -b2) + acc
            for cn, h0, h1 in (("a", h[0], h[1]), ("b", h[2], h[3])):
                for part in range(2):
                    mslice = slice(0, 128) if part == 0 else slice(128, DPHI)
                    mp = 128 if part == 0 else DPHI - 128
                    ppt = pp.tile([mp, NT], f32, name="pp2", tag=f"pp2_{part}", bufs=2)
                    nc.tensor.matmul(ppt[:], w2[cn, 0][:, mslice], h0[:], start=True, stop=False)
                    nc.tensor.matmul(ppt[:], w2[cn, 1][:, mslice], h1[:], start=False, stop=True)
                    a = acc[cn][part]
                    nb = nb2[cn][part]
                    if o == 0:
                        nc.vector.tensor_scalar(a[:], ppt[:], nb[:], None, ALU.max)
                    else:
                        nc.vector.scalar_tensor_tensor(
                            a[:], ppt[:], nb[:], a[:], op0=ALU.max, op1=ALU.add
                        )

        # rho: q_c = relu(p @ R1 + rb1') @ R2 + rb2
        for ci, cn in enumerate(("a", "b")):
            a0, a1 = acc[cn]
            t0, t1 = rw1[cn]
            zr = []
            for m in range(2):
                ms = slice(m * 128, (m + 1) * 128)
                psz = pp.tile([128, NT], f32, name="psz", tag="ph", bufs=4)
                nc.tensor.matmul(psz[:], t0[:, ms], a0[:], start=True, stop=False)
                nc.tensor.matmul(psz[:], t1[:, ms], a1[:], start=False, stop=True)
                zt = dp.tile([128, NT], bf16, name="zr", tag="zr", bufs=4)
                nc.scalar.activation(zt[:], psz[:], AF.Relu, bias=rb1c[2 * ci + m][:])
                zr.append(zt)
            psq = pp.tile([1, NT], f32, name="psq", tag="pp2_1", bufs=2)
            nc.tensor.matmul(psq[:], rw2[cn][0][:], zr[0][:], start=True, stop=False)
            nc.tensor.matmul(psq[:], rw2[cn][1][:], zr[1][:], start=False, stop=True)
            qt = dp.tile([1, NT], f32, name="qt", tag="qt", bufs=4)
            nc.scalar.activation(qt[:], psq[:], AF.Identity, bias=rb2c[ci][:])
            qd = q1_d if ci == 0 else q2_d
            nc.sync.dma_start(qd[bs, :], qt[:])


def prep_inputs(inputs, S, core):
    """Host-side shard + layout prep for one core. Returns the in_map."""
    import ml_dtypes

    bf = ml_dtypes.bfloat16
    lo, hi = core * S, (core + 1) * S
    obs = inputs["obs"][lo:hi]
    act = inputs["act"][lo:hi]
    ctx = inputs["context_layer"][lo:hi]
    body_t = np.ascontiguousarray(
        np.concatenate([obs[:, :DIM_BODY], act], axis=1).T
    ).astype(bf)
    ctx_t = np.ascontiguousarray(np.transpose(ctx, (1, 2, 0))).astype(bf)

    w1s = np.concatenate([inputs["phi_w1a"], inputs["phi_w1b"]], axis=1)
    b1s = np.concatenate([inputs["phi_b1a"], inputs["phi_b1b"]]).reshape(4, 128)
    b2a, b2b = inputs["phi_b2a"], inputs["phi_b2b"]
    nb2_128 = np.stack([-b2a[0:128], -b2b[0:128]])
    nb2_64 = np.stack([-b2a[128:DPHI], -b2b[128:DPHI]])
    # rho1 bias adjusted for the +O*b2 correction of the max-based object sum
    rb1a = inputs["rho_b1a"] + O * (b2a @ inputs["rho_w1a"])
    rb1b = inputs["rho_b1b"] + O * (b2b @ inputs["rho_w1b"])
    rb1adj = np.concatenate([rb1a, rb1b]).reshape(4, 128)
    rb2 = np.stack([inputs["rho_b2a"], inputs["rho_b2b"]]).reshape(2, 1)

    f = np.float32
    return {
        "body_t": body_t,
        "ctx_t": ctx_t,
        "w1s": np.ascontiguousarray(w1s).astype(bf),
        "w2a": np.ascontiguousarray(inputs["phi_w2a"]).astype(bf),
        "w2b": np.ascontiguousarray(inputs["phi_w2b"]).astype(bf),
        "rw1a": np.ascontiguousarray(inputs["rho_w1a"]).astype(bf),
        "rw1b": np.ascontiguousarray(inputs["rho_w1b"]).astype(bf),
        "rw2a": np.ascontiguousarray(inputs["rho_w2a"]).astype(bf),
        "rw2b": np.ascontiguousarray(inputs["rho_w2b"]).astype(bf),
        "b1s": np.ascontiguousarray(b1s, f),
        "nb2_128": np.ascontiguousarray(nb2_128, f),
        "nb2_64": np.ascontiguousarray(nb2_64, f),
        "rb1adj": np.ascontiguousarray(rb1adj, f),
        "rb2": np.ascontiguousarray(rb2, f),
    }


_CACHE = {}


def kernel(**inputs):
    from concourse.bass_utils import run_bass_kernel_spmd

    B = inputs["obs"].shape[0]
    assert B % N_CORES == 0
    S = B // N_CORES

    if S not in _CACHE:
        _CACHE[S] = build_nc(S)
    nc = _CACHE[S]

    in_maps = [prep_inputs(inputs, S, c) for c in range(N_CORES)]
    res = run_bass_kernel_spmd(nc, in_maps, list(range(N_CORES)))
    q1 = np.concatenate([res.results[c]["q1"] for c in range(N_CORES)], axis=0)
    q2 = np.concatenate([res.results[c]["q2"] for c in range(N_CORES)], axis=0)
    return (q1.astype(np.float32), q2.astype(np.float32))


if __name__ == "__main__":
    # smoke test with random data
    rng = np.random.default_rng(0)
    B = 32768
    ins = {
        "obs": rng.standard_normal((B, 100), dtype=np.float32),
        "act": rng.standard_normal((B, DIM_ACT), dtype=np.float32),
        "context_layer": rng.standard_normal((B, O, DCTX), dtype=np.float32),
    }
    for n, shp in (
        ("phi_w1a", (264, 256)), ("phi_b1a", (256,)),
        ("phi_w2a", (256, 192)), ("phi_b2a", (192,)),
        ("phi_w1b", (264, 256)), ("phi_b1b", (256,)),
        ("phi_w2b", (256, 192)), ("phi_b2b", (192,)),
        ("rho_w1a", (192, 256)), ("rho_b1a", (256,)),
        ("rho_w2a", (256, 1)), ("rho_b2a", (1,)),
        ("rho_w1b", (192, 256)), ("rho_b1b", (256,)),
        ("rho_w2b", (256, 1)), ("rho_b2b", (1,)),
    ):
        ins[n] = (rng.standard_normal(shp) * 0.05).astype(np.float32)
    q1, q2 = kernel(**ins)
    print(q1.shape, q2.shape, q1[:4, 0], q2[:4, 0])
